# revision 7
# baseline (speedup 1.0000x reference)
"""Trainium2 Bass kernel for KDPointToPointLoss (exact 1-NN + MSE).

Math: loss = mean_b mean_{n,d} ||s_n - t_{nn(n)}||^2
           = (1/(B*N*3)) * sum_{b,n} min_m ||s_n - t_m||^2
so only the min distance VALUES are needed (no argmin indices / gather).

Device computes, per source row n:  min_m ( s2[n] + t2[m] - 2*s_n.t_m )
via a K=24 bf16 matmul (hi/lo/lo2 splits of s, t, s2, t2 keep fp32-level
accuracy; matmul time on the PE depends only on the moving free dim, not K)
followed by a tensor_tensor_reduce(min,min) fold on the Vector engine.

Sharding: 8 cores; core = b*4 + q owns batch b, source rows q*2048..(q+1)*2048,
and sees the full target cloud of its batch. Final scalar reduction on host.
"""

import os
import numpy as np
import ml_dtypes

import concourse.bass as bass
import concourse.bacc as bacc
import concourse.mybir as mybir
from concourse.tile import TileContext
from concourse.bass_utils import run_bass_kernel_spmd

bf16 = ml_dtypes.bfloat16

B, N, M, D = 2, 8192, 8192, 3
N_CORES = 8
CORES_PER_BATCH = N_CORES // B          # 4
N_SHARD = N // CORES_PER_BATCH          # 2048 source rows per core
N_TILES = N_SHARD // 128                # 16 tiles of 128 rows
M_CHUNK = 512
M_CHUNKS = M // M_CHUNK                 # 16
GROUP = 2048                            # PSUM tile: 4 banks, reduced in one op
M_GROUPS = M // GROUP                   # 4 reduce groups per tile row
K = 24

_BIG = 3.0e38


def _split3(x):
    """fp64 array -> (hi, lo, lo2) bf16 triple with residual ~2^-24."""
    x = x.astype(np.float64)
    h = x.astype(bf16)
    r = x - h.astype(np.float64)
    l = r.astype(bf16)
    r2 = r - l.astype(np.float64)
    l2 = r2.astype(bf16)
    return h, l, l2


def _build_bass():
    nc = bacc.Bacc(trn_type="TRN2")
    lhs_d = nc.dram_tensor("lhs", [K, N_SHARD], mybir.dt.bfloat16, kind="ExternalInput")
    rhs_d = nc.dram_tensor("rhs", [K, M], mybir.dt.bfloat16, kind="ExternalInput")
    out_d = nc.dram_tensor("out", [128, N_TILES], mybir.dt.float32, kind="ExternalOutput")

    fp32 = mybir.dt.float32
    mn = mybir.AluOpType.min

    with TileContext(nc) as tc:
        with (
            tc.tile_pool(name="const", bufs=1) as cpool,
            tc.tile_pool(name="psum", bufs=2, space="PSUM") as ppool,
        ):
            lhs_sb = cpool.tile([K, N_SHARD], mybir.dt.bfloat16)
            rhs_sb = cpool.tile([K, M], mybir.dt.bfloat16)
            acc = cpool.tile([128, N_TILES * M_GROUPS], fp32)
            accf = cpool.tile([128, N_TILES], fp32)

            nc.sync.dma_start(lhs_sb[:], lhs_d[:])
            nc.sync.dma_start(rhs_sb[:], rhs_d[:])

            for t in range(N_TILES):
                lhsT = lhs_sb[:, t * 128:(t + 1) * 128]
                for g in range(M_GROUPS):
                    ps = ppool.tile([128, GROUP], fp32, tag="ps")
                    for j in range(GROUP // M_CHUNK):
                        c = g * GROUP + j * M_CHUNK
                        nc.tensor.matmul(
                            ps[:, j * M_CHUNK:(j + 1) * M_CHUNK],
                            lhsT, rhs_sb[:, c:c + M_CHUNK],
                            start=True, stop=True)
                    nc.vector.tensor_reduce(
                        out=acc[:, t * M_GROUPS + g: t * M_GROUPS + g + 1],
                        in_=ps[:], axis=mybir.AxisListType.X, op=mn,
                    )

            nc.vector.tensor_reduce(
                out=accf[:],
                in_=acc[:].rearrange("q (t g) -> q t g", g=M_GROUPS),
                axis=mybir.AxisListType.X, op=mn,
            )
            nc.sync.dma_start(out_d[:], accf[:])
    nc.finalize()
    return nc


_NC_CACHE = None


def _get_nc():
    global _NC_CACHE
    if _NC_CACHE is None:
        _NC_CACHE = _build_bass()
    return _NC_CACHE


def _prepare_inputs(source_point_cloud, target_point_cloud):
    """Host-side: bf16 hi/lo/lo2 splits + per-core operand layout."""
    s_all = np.asarray(source_point_cloud, dtype=np.float32)
    t_all = np.asarray(target_point_cloud, dtype=np.float32)

    in_maps = []
    s2_resid = []  # per-core fp64 residual sum: sum_n (s2 - s2_dev)
    for core in range(N_CORES):
        b = core // CORES_PER_BATCH
        q = core % CORES_PER_BATCH
        s = s_all[b, q * N_SHARD:(q + 1) * N_SHARD, :]
        t = t_all[b]

        sh, sl, sl2 = _split3(s)
        th, tl, tl2 = _split3(t)
        s2 = (s.astype(np.float64) ** 2).sum(-1)
        t2 = (t.astype(np.float64) ** 2).sum(-1)
        s2h, s2l, s2l2 = _split3(s2)
        t2h, t2l, t2l2 = _split3(t2)

        lhs = np.zeros((K, N_SHARD), dtype=bf16)
        rhs = np.zeros((K, M), dtype=bf16)
        ones_n = np.ones(N_SHARD, dtype=bf16)
        ones_m = np.ones(M, dtype=bf16)

        def m2(x):  # -2x, exact in bf16
            return (np.float32(-2.0) * x.astype(np.float32)).astype(bf16)

        for d in range(D):
            lhs[0 + d] = sh[:, d];  rhs[0 + d] = m2(th[:, d])
            lhs[3 + d] = sh[:, d];  rhs[3 + d] = m2(tl[:, d])
            lhs[6 + d] = sl[:, d];  rhs[6 + d] = m2(th[:, d])
            lhs[9 + d] = sl[:, d];  rhs[9 + d] = m2(tl[:, d])
            lhs[12 + d] = sh[:, d]; rhs[12 + d] = m2(tl2[:, d])
            lhs[15 + d] = sl2[:, d]; rhs[15 + d] = m2(th[:, d])
        lhs[18] = ones_n; rhs[18] = t2h
        lhs[19] = ones_n; rhs[19] = t2l
        lhs[20] = ones_n; rhs[20] = t2l2
        lhs[21] = s2h;    rhs[21] = ones_m
        lhs[22] = s2l;    rhs[22] = ones_m
        lhs[23] = s2l2;   rhs[23] = ones_m

        s2_dev = (s2h.astype(np.float64) + s2l.astype(np.float64)
                  + s2l2.astype(np.float64))
        s2_resid.append((s2 - s2_dev).sum())
        in_maps.append({"lhs": lhs, "rhs": rhs})

    return in_maps, s2_resid


def _run(source_point_cloud, target_point_cloud, trace=False):
    in_maps, s2_resid = _prepare_inputs(source_point_cloud, target_point_cloud)
    nc = _get_nc()
    res = run_bass_kernel_spmd(nc, in_maps, core_ids=list(range(N_CORES)),
                               trace=trace)
    total = 0.0
    for core in range(N_CORES):
        out = res.results[core]["out"]  # [128, N_TILES] fp32; row n = t*128 + p
        total += out.astype(np.float64).sum() + s2_resid[core]
    loss = total / (B * N * D)
    return np.float32(loss), res


def kernel(source_point_cloud, target_point_cloud):
    out, _ = _run(source_point_cloud, target_point_cloud,
                  trace=bool(os.environ.get("BASS_TRACE")))
    return out


# revision 9
# speedup vs baseline: 1.1675x; 1.1675x over previous
"""Trainium2 Bass kernel for KDPointToPointLoss (exact 1-NN + MSE).

Math: loss = mean_b mean_{n,d} ||s_n - t_{nn(n)}||^2
           = (1/(B*N*3)) * sum_{b,n} min_m ||s_n - t_m||^2
so only the min distance VALUES are needed (no argmin indices / gather).

Device computes, per source row n:  min_m ( s2[n] + t2[m] - 2*s_n.t_m )
via a K=24 bf16 matmul (hi/lo/lo2 splits of s, t, s2, t2 give fp32-level
accuracy; PE matmul time depends only on the moving free dim, not K),
then folds the min on the Vector engine with a custom 2-input DVE op
(min body + min accumulate = 2 elements/cycle, 2x the native reduce).

PE packing: weights are replicated into two 32-row groups (partition base
0 and 32); matmuls alternate row groups so each LDWEIGHTS overlaps the
other group's in-flight matmul (same-group LDW+MM serialize otherwise).

Sharding: 8 cores; core = b*4 + q owns batch b, source rows q*2048..(q+1)*2048,
full target cloud of its batch. Final scalar reduction on host.
"""

import os
import numpy as np
import ml_dtypes

import concourse.bass as bass
import concourse.bacc as bacc
import concourse.mybir as mybir
from concourse.tile import TileContext
from concourse.bass_utils import run_bass_kernel_spmd

bf16 = ml_dtypes.bfloat16

B, N, M, D = 2, 8192, 8192, 3
N_CORES = 8
CORES_PER_BATCH = N_CORES // B          # 4
N_SHARD = N // CORES_PER_BATCH          # 2048 source rows per core
N_TILES = N_SHARD // 128                # 16 tiles of 128 rows
M_CHUNK = 512                           # one matmul / one PSUM bank
GROUP = 2048                            # PSUM tile: 4 banks, one DVE fold op
M_GROUPS = M // GROUP                   # 4 fold groups per tile row
K = 24

_BIG = 3.0e38


# ---------------------------------------------------------------- custom DVE op
_MIN2 = None


def _get_min2_op():
    """Register MIN2_REDUCE_ANT: out = min(in0, in1); accum = min(s0, min(out)).
    Reads 2 tensor streams at 1 elem/cycle each -> 2x native tensor_reduce."""
    global _MIN2
    if _MIN2 is not None:
        return _MIN2
    import concourse.dve_ops as dve_ops
    from concourse.dve_spec import Spec, Src0, Src1, C0, minn, lower, _has_src1
    from concourse.dve_uop import DveOpSpec

    for op in dve_ops.OPS:
        if op.name == "MIN2_REDUCE_ANT":
            _MIN2 = op
            return op

    def _ref(in0, in1, c0, c1, c2):
        b = np.minimum(in0.astype(np.float32), in1.astype(np.float32))
        acc = np.minimum(
            np.minimum.reduce(b.reshape(b.shape[0], -1), axis=-1, keepdims=True),
            np.asarray(c0, np.float32).reshape(-1, 1))
        return b, acc

    spec = Spec(body=minn(Src0, Src1), accum=minn, accum_init=C0, reference=_ref)
    opcode = dve_ops._CUSTOM_DVE_ROW_BASE + len(dve_ops.OPS)
    sha = {}
    for ver in ("v3", "v4"):
        uops = lower(spec, ver=ver)
        sha[ver] = DveOpSpec(name="MIN2_REDUCE_ANT", opcode=opcode, uops=uops,
                             rd1_en=_has_src1(spec)).sha(ver)
    op = dve_ops.DveOp("MIN2_REDUCE_ANT", spec, subdim=False, uops_sha=sha)
    dve_ops.OPS.append(op)
    dve_ops._SUB_OPCODE_FOR_NAME[op.name] = opcode
    _MIN2 = op
    return op


def _split3(x):
    """fp64 array -> (hi, lo, lo2) bf16 triple with residual ~2^-24."""
    x = x.astype(np.float64)
    h = x.astype(bf16)
    r = x - h.astype(np.float64)
    l = r.astype(bf16)
    r2 = r - l.astype(np.float64)
    l2 = r2.astype(bf16)
    return h, l, l2


def _build_bass():
    min2 = _get_min2_op()
    nc = bacc.Bacc(trn_type="TRN2")
    # row-group replicas live at partition bases 0 and 32
    lhs_d = nc.dram_tensor("lhs", [64, N_SHARD], mybir.dt.bfloat16, kind="ExternalInput")
    rhs_d = nc.dram_tensor("rhs", [64, M], mybir.dt.bfloat16, kind="ExternalInput")
    out_d = nc.dram_tensor("out", [128, N_TILES], mybir.dt.float32, kind="ExternalOutput")

    fp32 = mybir.dt.float32
    mn = mybir.AluOpType.min

    with TileContext(nc) as tc:
        with (
            tc.tile_pool(name="const", bufs=1) as cpool,
            tc.tile_pool(name="psum", bufs=2, space="PSUM") as ppool,
            tc.tile_pool(name="scratch", bufs=2) as spool,
        ):
            lhs_sb = cpool.tile([64, N_SHARD], mybir.dt.bfloat16)
            rhs_sb = cpool.tile([64, M], mybir.dt.bfloat16)
            acc = cpool.tile([128, N_TILES * M_GROUPS], fp32)
            accf = cpool.tile([128, N_TILES], fp32)

            nc.sync.dma_start(lhs_sb[:], lhs_d[:])
            nc.sync.dma_start(rhs_sb[:], rhs_d[:])

            for t in range(N_TILES):
                lhsT = {rg: lhs_sb[32 * rg:32 * rg + K, t * 128:(t + 1) * 128]
                        for rg in (0, 1)}
                for g in range(M_GROUPS):
                    ps = ppool.tile([128, GROUP], fp32, tag="ps")
                    for j in range(GROUP // M_CHUNK):
                        rg = j % 2          # alternate row groups -> LDW overlaps MM
                        c = g * GROUP + j * M_CHUNK
                        nc.tensor.matmul(
                            ps[:, j * M_CHUNK:(j + 1) * M_CHUNK],
                            lhsT[rg],
                            rhs_sb[32 * rg:32 * rg + K, c:c + M_CHUNK],
                            start=True, stop=True)
                    # only one DVE input may be PSUM: ScalarE stages the
                    # second half into SBUF while DVE reads the first
                    half = spool.tile([128, GROUP // 2], fp32, tag="half")
                    nc.scalar.copy(half[:], ps[:, GROUP // 2:])
                    scr = spool.tile([128, GROUP // 2], fp32, tag="scr")
                    nc.vector._custom_dve(
                        min2,
                        out=scr[:],
                        in0=ps[:, :GROUP // 2],
                        in1=half[:],
                        s0=_BIG,
                        accum_out=acc[:, t * M_GROUPS + g: t * M_GROUPS + g + 1],
                    )

            nc.vector.tensor_reduce(
                out=accf[:],
                in_=acc[:].rearrange("q (t g) -> q t g", g=M_GROUPS),
                axis=mybir.AxisListType.X, op=mn,
            )
            nc.sync.dma_start(out_d[:], accf[:])
    nc.finalize()
    return nc


_NC_CACHE = None


def _get_nc():
    global _NC_CACHE
    if _NC_CACHE is None:
        _NC_CACHE = _build_bass()
    return _NC_CACHE


def _prepare_inputs(source_point_cloud, target_point_cloud):
    """Host-side: bf16 hi/lo/lo2 splits + per-core operand layout."""
    s_all = np.asarray(source_point_cloud, dtype=np.float32)
    t_all = np.asarray(target_point_cloud, dtype=np.float32)

    # rhs is shared per batch: build once per batch
    rhs_by_batch = []
    for b in range(B):
        t = t_all[b]
        th, tl, tl2 = _split3(t)
        t2 = (t.astype(np.float64) ** 2).sum(-1)
        t2h, t2l, t2l2 = _split3(t2)
        rhs = np.zeros((64, M), dtype=bf16)
        ones_m = np.ones(M, dtype=bf16)

        def m2(x):  # -2x, exact in bf16
            return (np.float32(-2.0) * x.astype(np.float32)).astype(bf16)

        for d in range(D):
            rhs[0 + d] = m2(th[:, d])
            rhs[3 + d] = m2(tl[:, d])
            rhs[6 + d] = m2(th[:, d])
            rhs[9 + d] = m2(tl[:, d])
            rhs[12 + d] = m2(tl2[:, d])
            rhs[15 + d] = m2(th[:, d])
        rhs[18] = t2h
        rhs[19] = t2l
        rhs[20] = t2l2
        rhs[21] = ones_m
        rhs[22] = ones_m
        rhs[23] = ones_m
        rhs[32:32 + K] = rhs[0:K]          # row-group 1 replica
        rhs_by_batch.append(rhs)

    in_maps = []
    s2_resid = []
    for core in range(N_CORES):
        b = core // CORES_PER_BATCH
        q = core % CORES_PER_BATCH
        s = s_all[b, q * N_SHARD:(q + 1) * N_SHARD, :]

        sh, sl, sl2 = _split3(s)
        s2 = (s.astype(np.float64) ** 2).sum(-1)
        s2h, s2l, s2l2 = _split3(s2)

        lhs = np.zeros((64, N_SHARD), dtype=bf16)
        ones_n = np.ones(N_SHARD, dtype=bf16)
        for d in range(D):
            lhs[0 + d] = sh[:, d]
            lhs[3 + d] = sh[:, d]
            lhs[6 + d] = sl[:, d]
            lhs[9 + d] = sl[:, d]
            lhs[12 + d] = sh[:, d]
            lhs[15 + d] = sl2[:, d]
        lhs[18] = ones_n
        lhs[19] = ones_n
        lhs[20] = ones_n
        lhs[21] = s2h
        lhs[22] = s2l
        lhs[23] = s2l2
        lhs[32:32 + K] = lhs[0:K]          # row-group 1 replica

        s2_dev = (s2h.astype(np.float64) + s2l.astype(np.float64)
                  + s2l2.astype(np.float64))
        s2_resid.append((s2 - s2_dev).sum())
        in_maps.append({"lhs": lhs, "rhs": rhs_by_batch[b]})

    return in_maps, s2_resid


def _run(source_point_cloud, target_point_cloud, trace=False):
    in_maps, s2_resid = _prepare_inputs(source_point_cloud, target_point_cloud)
    nc = _get_nc()
    res = run_bass_kernel_spmd(nc, in_maps, core_ids=list(range(N_CORES)),
                               trace=trace)
    total = 0.0
    for core in range(N_CORES):
        out = res.results[core]["out"]  # [128, N_TILES]; row n = t*128 + p
        total += out.astype(np.float64).sum() + s2_resid[core]
    loss = total / (B * N * D)
    return np.float32(loss), res


def kernel(source_point_cloud, target_point_cloud):
    out, _ = _run(source_point_cloud, target_point_cloud,
                  trace=bool(os.environ.get("BASS_TRACE")))
    return out


# revision 11
# speedup vs baseline: 1.6036x; 1.3736x over previous
"""Trainium2 Bass kernel for KDPointToPointLoss (exact 1-NN + MSE).

Math: loss = mean_b mean_{n,d} ||s_n - t_{nn(n)}||^2
           = (1/(B*N*3)) * sum_{b,n} min_m ||s_n - t_m||^2
so only the min distance VALUES are needed (no argmin indices / gather).

Device computes, per source row n:  min_m ( s2[n] + t2[m] - 2*s_n.t_m )
via a K=24 bf16 matmul (hi/lo/lo2 splits of s, t, s2, t2 give fp32-level
accuracy; PE matmul time depends only on the moving free dim, not K),
then folds the min on the Vector engine with a custom 2-input DVE op
(min body + min accumulate = 2 elements/cycle, 2x the native reduce).

PE packing: weights are replicated into two 32-row groups (partition base
0 and 32); matmuls alternate row groups so each LDWEIGHTS overlaps the
other group's in-flight matmul (same-group LDW+MM serialize otherwise).

Sharding: 8 cores; core = b*4 + q owns batch b, source rows q*2048..(q+1)*2048,
full target cloud of its batch. Final scalar reduction on host.
"""

import os
import numpy as np
import ml_dtypes

import concourse.bass as bass
import concourse.bacc as bacc
import concourse.mybir as mybir
from concourse.tile import TileContext
from concourse.bass_utils import run_bass_kernel_spmd

bf16 = ml_dtypes.bfloat16

B, N, M, D = 2, 8192, 8192, 3
N_CORES = 8
CORES_PER_BATCH = N_CORES // B          # 4
N_SHARD = N // CORES_PER_BATCH          # 2048 source rows per core
N_TILES = N_SHARD // 128                # 16 tiles of 128 rows
M_CHUNK = 512                           # one matmul / one PSUM bank
GROUP = 1024                            # PSUM tile: 2 banks, one DVE fold op
M_GROUPS = M // GROUP                   # fold groups per tile row
K = 24

_BIG = 3.0e38


# ---------------------------------------------------------------- custom DVE op
_MIN2 = None


def _get_min2_op():
    """Register MIN2_REDUCE_ANT: out = min(in0, in1); accum = min(s0, min(out)).
    Reads 2 tensor streams at 1 elem/cycle each -> 2x native tensor_reduce."""
    global _MIN2
    if _MIN2 is not None:
        return _MIN2
    import concourse.dve_ops as dve_ops
    from concourse.dve_spec import Spec, Src0, Src1, C0, minn, lower, _has_src1
    from concourse.dve_uop import DveOpSpec

    for op in dve_ops.OPS:
        if op.name == "MIN2_REDUCE_ANT":
            _MIN2 = op
            return op

    def _ref(in0, in1, c0, c1, c2):
        b = np.minimum(in0.astype(np.float32), in1.astype(np.float32))
        acc = np.minimum(
            np.minimum.reduce(b.reshape(b.shape[0], -1), axis=-1, keepdims=True),
            np.asarray(c0, np.float32).reshape(-1, 1))
        return b, acc

    spec = Spec(body=minn(Src0, Src1), accum=minn, accum_init=C0, reference=_ref)
    opcode = dve_ops._CUSTOM_DVE_ROW_BASE + len(dve_ops.OPS)
    sha = {}
    for ver in ("v3", "v4"):
        uops = lower(spec, ver=ver)
        sha[ver] = DveOpSpec(name="MIN2_REDUCE_ANT", opcode=opcode, uops=uops,
                             rd1_en=_has_src1(spec)).sha(ver)
    op = dve_ops.DveOp("MIN2_REDUCE_ANT", spec, subdim=False, uops_sha=sha)
    dve_ops.OPS.append(op)
    dve_ops._SUB_OPCODE_FOR_NAME[op.name] = opcode
    _MIN2 = op
    return op


def _split3(x):
    """fp64 array -> (hi, lo, lo2) bf16 triple with residual ~2^-24."""
    x = x.astype(np.float64)
    h = x.astype(bf16)
    r = x - h.astype(np.float64)
    l = r.astype(bf16)
    r2 = r - l.astype(np.float64)
    l2 = r2.astype(bf16)
    return h, l, l2


def _build_bass():
    min2 = _get_min2_op()
    nc = bacc.Bacc(trn_type="TRN2")
    # row-group replicas live at partition bases 0 and 32
    lhs_d = nc.dram_tensor("lhs", [64, N_SHARD], mybir.dt.bfloat16, kind="ExternalInput")
    rhs_d = nc.dram_tensor("rhs", [64, M], mybir.dt.bfloat16, kind="ExternalInput")
    out_d = nc.dram_tensor("out", [128, N_TILES], mybir.dt.float32, kind="ExternalOutput")

    fp32 = mybir.dt.float32
    mn = mybir.AluOpType.min

    with TileContext(nc) as tc:
        with (
            tc.tile_pool(name="const", bufs=1) as cpool,
            tc.tile_pool(name="psum", bufs=4, space="PSUM") as ppool,
            tc.tile_pool(name="scratch", bufs=4) as spool,
        ):
            lhs_sb = cpool.tile([64, N_SHARD], mybir.dt.bfloat16)
            rhs_sb = cpool.tile([64, M], mybir.dt.bfloat16)
            acc = cpool.tile([128, N_TILES * M_GROUPS], fp32)
            accf = cpool.tile([128, N_TILES], fp32)

            nc.sync.dma_start(lhs_sb[:], lhs_d[:])
            nc.sync.dma_start(rhs_sb[:], rhs_d[:])

            for t in range(N_TILES):
                lhsT = {rg: lhs_sb[32 * rg:32 * rg + K, t * 128:(t + 1) * 128]
                        for rg in (0, 1)}
                for g in range(M_GROUPS):
                    ps = ppool.tile([128, GROUP], fp32, tag="ps")
                    for j in range(GROUP // M_CHUNK):
                        rg = j % 2          # alternate row groups -> LDW overlaps MM
                        c = g * GROUP + j * M_CHUNK
                        nc.tensor.matmul(
                            ps[:, j * M_CHUNK:(j + 1) * M_CHUNK],
                            lhsT[rg],
                            rhs_sb[32 * rg:32 * rg + K, c:c + M_CHUNK],
                            start=True, stop=True)
                    # only one DVE input may be PSUM: ScalarE stages the
                    # second half into SBUF while DVE reads the first
                    half = spool.tile([128, GROUP // 2], fp32, tag="half")
                    nc.scalar.copy(half[:], ps[:, GROUP // 2:])
                    scr = spool.tile([128, GROUP // 2], fp32, tag="scr")
                    nc.vector._custom_dve(
                        min2,
                        out=scr[:],
                        in0=ps[:, :GROUP // 2],
                        in1=half[:],
                        s0=_BIG,
                        accum_out=acc[:, t * M_GROUPS + g: t * M_GROUPS + g + 1],
                    )

            nc.vector.tensor_reduce(
                out=accf[:],
                in_=acc[:].rearrange("q (t g) -> q t g", g=M_GROUPS),
                axis=mybir.AxisListType.X, op=mn,
            )
            nc.sync.dma_start(out_d[:], accf[:])
    nc.finalize()
    return nc


_NC_CACHE = None


def _get_nc():
    global _NC_CACHE
    if _NC_CACHE is None:
        _NC_CACHE = _build_bass()
    return _NC_CACHE


def _prepare_inputs(source_point_cloud, target_point_cloud):
    """Host-side: bf16 hi/lo/lo2 splits + per-core operand layout."""
    s_all = np.asarray(source_point_cloud, dtype=np.float32)
    t_all = np.asarray(target_point_cloud, dtype=np.float32)

    # rhs is shared per batch: build once per batch
    rhs_by_batch = []
    for b in range(B):
        t = t_all[b]
        th, tl, tl2 = _split3(t)
        t2 = (t.astype(np.float64) ** 2).sum(-1)
        t2h, t2l, t2l2 = _split3(t2)
        rhs = np.zeros((64, M), dtype=bf16)
        ones_m = np.ones(M, dtype=bf16)

        def m2(x):  # -2x, exact in bf16
            return (np.float32(-2.0) * x.astype(np.float32)).astype(bf16)

        for d in range(D):
            rhs[0 + d] = m2(th[:, d])
            rhs[3 + d] = m2(tl[:, d])
            rhs[6 + d] = m2(th[:, d])
            rhs[9 + d] = m2(tl[:, d])
            rhs[12 + d] = m2(tl2[:, d])
            rhs[15 + d] = m2(th[:, d])
        rhs[18] = t2h
        rhs[19] = t2l
        rhs[20] = t2l2
        rhs[21] = ones_m
        rhs[22] = ones_m
        rhs[23] = ones_m
        rhs[32:32 + K] = rhs[0:K]          # row-group 1 replica
        rhs_by_batch.append(rhs)

    in_maps = []
    s2_resid = []
    for core in range(N_CORES):
        b = core // CORES_PER_BATCH
        q = core % CORES_PER_BATCH
        s = s_all[b, q * N_SHARD:(q + 1) * N_SHARD, :]

        sh, sl, sl2 = _split3(s)
        s2 = (s.astype(np.float64) ** 2).sum(-1)
        s2h, s2l, s2l2 = _split3(s2)

        lhs = np.zeros((64, N_SHARD), dtype=bf16)
        ones_n = np.ones(N_SHARD, dtype=bf16)
        for d in range(D):
            lhs[0 + d] = sh[:, d]
            lhs[3 + d] = sh[:, d]
            lhs[6 + d] = sl[:, d]
            lhs[9 + d] = sl[:, d]
            lhs[12 + d] = sh[:, d]
            lhs[15 + d] = sl2[:, d]
        lhs[18] = ones_n
        lhs[19] = ones_n
        lhs[20] = ones_n
        lhs[21] = s2h
        lhs[22] = s2l
        lhs[23] = s2l2
        lhs[32:32 + K] = lhs[0:K]          # row-group 1 replica

        s2_dev = (s2h.astype(np.float64) + s2l.astype(np.float64)
                  + s2l2.astype(np.float64))
        s2_resid.append((s2 - s2_dev).sum())
        in_maps.append({"lhs": lhs, "rhs": rhs_by_batch[b]})

    return in_maps, s2_resid


def _run(source_point_cloud, target_point_cloud, trace=False):
    in_maps, s2_resid = _prepare_inputs(source_point_cloud, target_point_cloud)
    nc = _get_nc()
    res = run_bass_kernel_spmd(nc, in_maps, core_ids=list(range(N_CORES)),
                               trace=trace)
    total = 0.0
    for core in range(N_CORES):
        out = res.results[core]["out"]  # [128, N_TILES]; row n = t*128 + p
        total += out.astype(np.float64).sum() + s2_resid[core]
    loss = total / (B * N * D)
    return np.float32(loss), res


def kernel(source_point_cloud, target_point_cloud):
    out, _ = _run(source_point_cloud, target_point_cloud,
                  trace=bool(os.environ.get("BASS_TRACE")))
    return out


# revision 12
# speedup vs baseline: 4.6103x; 2.8749x over previous
"""Trainium2 Bass kernel for KDPointToPointLoss (exact 1-NN + MSE).

Math: loss = mean_b mean_{n,d} ||s_n - t_{nn(n)}||^2
           = (1/(B*N*3)) * sum_{b,n} min_m ||s_n - t_m||^2
so only the min distance VALUES are needed (no argmin indices / gather).

Exact norm-window pruning: sort sources and targets by radius (the loss is
permutation invariant). For a source tile (128 radius-adjacent sources) with
radius range [a,b] and a certified upper bound W >= max_n sqrt(min-dist_n),
every nearest neighbor lies among targets with radius in [a-W, b+W]: any
other target m has d2 >= (|t_m|-|s_n|)^2 > W^2 >= min-dist. W comes from a
cheap host scan of k rank-adjacent candidates (valid upper bound; the device
still evaluates every certified candidate exactly). This prunes ~85% of the
distance matrix on random clouds.

Device work = flat list of groups (source tile x 1024 gathered target cols):
K=24 bf16 matmul (hi/lo/lo2 splits of s, t, s2, t2 -> fp32-level accuracy)
into PSUM, then a custom 2-input DVE op (min body + min accumulate,
2 elems/cycle) folds each group to one accumulator column. ScalarE stages
half of each group PSUM->SBUF (DVE may read only one PSUM operand).
Matmuls alternate two row-group weight replicas so LDWEIGHTS overlaps the
other group's in-flight matmul. Host min-combines group columns (fp64).

Sharding: 8 cores; cores 0-3 batch 0, cores 4-7 batch 1, balanced by group
count; the gathered rhs keeps per-core inputs small.
"""

import os
import numpy as np
import ml_dtypes

import concourse.bass as bass
import concourse.bacc as bacc
import concourse.mybir as mybir
from concourse.tile import TileContext
from concourse.bass_utils import run_bass_kernel_spmd

bf16 = ml_dtypes.bfloat16

B, N, M, D = 2, 8192, 8192, 3
N_CORES = 8
CORES_PER_BATCH = N_CORES // B
M_CHUNK = 512
GROUP = 1024                 # columns per DVE fold group (2 PSUM banks)
K = 24
K_CAND = 256                 # host candidate scan width for upper bounds
_BIG = 3.0e38

_DMA_SPLIT = 8               # rhs arrives in pieces so compute starts early


# ---------------------------------------------------------------- custom DVE op
_MIN2 = None


def _get_min2_op():
    """MIN2_REDUCE_ANT: out = min(in0, in1); accum = min(s0, min(out)).
    Reads 2 tensor streams at 1 elem/cycle each -> 2x native tensor_reduce."""
    global _MIN2
    if _MIN2 is not None:
        return _MIN2
    import concourse.dve_ops as dve_ops
    from concourse.dve_spec import Spec, Src0, Src1, C0, minn, lower, _has_src1
    from concourse.dve_uop import DveOpSpec

    for op in dve_ops.OPS:
        if op.name == "MIN2_REDUCE_ANT":
            _MIN2 = op
            return op

    def _ref(in0, in1, c0, c1, c2):
        b = np.minimum(in0.astype(np.float32), in1.astype(np.float32))
        acc = np.minimum(
            np.minimum.reduce(b.reshape(b.shape[0], -1), axis=-1, keepdims=True),
            np.asarray(c0, np.float32).reshape(-1, 1))
        return b, acc

    spec = Spec(body=minn(Src0, Src1), accum=minn, accum_init=C0, reference=_ref)
    opcode = dve_ops._CUSTOM_DVE_ROW_BASE + len(dve_ops.OPS)
    sha = {}
    for ver in ("v3", "v4"):
        uops = lower(spec, ver=ver)
        sha[ver] = DveOpSpec(name="MIN2_REDUCE_ANT", opcode=opcode, uops=uops,
                             rd1_en=_has_src1(spec)).sha(ver)
    op = dve_ops.DveOp("MIN2_REDUCE_ANT", spec, subdim=False, uops_sha=sha)
    dve_ops.OPS.append(op)
    dve_ops._SUB_OPCODE_FOR_NAME[op.name] = opcode
    _MIN2 = op
    return op


def _split3(x):
    """fp64 array -> (hi, lo, lo2) bf16 triple with residual ~2^-24."""
    x = x.astype(np.float64)
    h = x.astype(bf16)
    r = x - h.astype(np.float64)
    l = r.astype(bf16)
    r2 = r - l.astype(np.float64)
    l2 = r2.astype(bf16)
    return h, l, l2


# ---------------------------------------------------------------- device kernel
_NC_CACHE = {}


def _build_bass(G):
    """Flat loop over G groups: 2 matmuls -> PSUM [128,1024], ScalarE stages
    the second half to SBUF, custom DVE op folds to acc[:, g]."""
    min2 = _get_min2_op()
    nc = bacc.Bacc(trn_type="TRN2")
    lhs_d = nc.dram_tensor("lhs", [64, G * 128], mybir.dt.bfloat16, kind="ExternalInput")
    rhs_d = nc.dram_tensor("rhs", [64, G * GROUP], mybir.dt.bfloat16, kind="ExternalInput")
    out_d = nc.dram_tensor("out", [128, G], mybir.dt.float32, kind="ExternalOutput")

    fp32 = mybir.dt.float32

    with TileContext(nc) as tc:
        with (
            tc.tile_pool(name="const", bufs=1) as cpool,
            tc.tile_pool(name="psum", bufs=4, space="PSUM") as ppool,
            tc.tile_pool(name="scratch", bufs=4) as spool,
        ):
            lhs_sb = cpool.tile([64, G * 128], mybir.dt.bfloat16)
            rhs_sb = cpool.tile([64, G * GROUP], mybir.dt.bfloat16)
            acc = cpool.tile([128, G], fp32)

            nc.sync.dma_start(lhs_sb[:], lhs_d[:])
            # split the rhs load so the first groups' matmuls start early
            step = (G + _DMA_SPLIT - 1) // _DMA_SPLIT
            for p in range(0, G, step):
                q = min(G, p + step)
                nc.sync.dma_start(rhs_sb[:, p * GROUP:q * GROUP],
                                  rhs_d[:, p * GROUP:q * GROUP])

            for g in range(G):
                lhsT = {rg: lhs_sb[32 * rg:32 * rg + K, g * 128:(g + 1) * 128]
                        for rg in (0, 1)}
                ps = ppool.tile([128, GROUP], fp32, tag="ps")
                for j in range(GROUP // M_CHUNK):
                    rg = j % 2      # alternate row groups -> LDW overlaps MM
                    c = g * GROUP + j * M_CHUNK
                    nc.tensor.matmul(
                        ps[:, j * M_CHUNK:(j + 1) * M_CHUNK],
                        lhsT[rg],
                        rhs_sb[32 * rg:32 * rg + K, c:c + M_CHUNK],
                        start=True, stop=True)
                # only one DVE input may be PSUM: ScalarE stages the second half
                half = spool.tile([128, GROUP // 2], fp32, tag="half")
                nc.scalar.copy(half[:], ps[:, GROUP // 2:])
                scr = spool.tile([128, GROUP // 2], fp32, tag="scr")
                nc.vector._custom_dve(
                    min2,
                    out=scr[:],
                    in0=ps[:, :GROUP // 2],
                    in1=half[:],
                    s0=_BIG,
                    accum_out=acc[:, g:g + 1],
                )

            nc.sync.dma_start(out_d[:], acc[:])
    nc.finalize()
    return nc


def _get_nc(G):
    if G not in _NC_CACHE:
        _NC_CACHE[G] = _build_bass(G)
    return _NC_CACHE[G]


# ---------------------------------------------------------------- host planning
def _plan_batch(s, t):
    """Sort by radius, certify per-tile target chunk windows (exact)."""
    s = s.astype(np.float64)
    t = t.astype(np.float64)
    n, m = len(s), len(t)
    sn = np.linalg.norm(s, axis=1)
    tn = np.linalg.norm(t, axis=1)
    so = np.argsort(sn, kind="stable")
    to = np.argsort(tn, kind="stable")
    s_s, sn_s = s[so], sn[so]
    t_s, tn_s = t[to], tn[to]

    # upper bound on each source's NN distance from k rank-adjacent candidates
    idx = np.searchsorted(tn_s, sn_s)
    lo = np.clip(idx - K_CAND // 2, 0, m - K_CAND)
    cand_idx = lo[:, None] + np.arange(K_CAND)[None, :]
    d2 = ((s_s[:, None, :] - t_s[cand_idx]) ** 2).sum(-1)
    ub = d2.min(1)

    W = np.sqrt(ub)
    ntiles = n // 128
    windows = []
    for ti in range(ntiles):
        sl = slice(ti * 128, (ti + 1) * 128)
        Wt = W[sl].max() * (1 + 1e-9) + 1e-12
        lo_t = np.searchsorted(tn_s, sn_s[sl].min() - Wt, side="left")
        hi_t = np.searchsorted(tn_s, sn_s[sl].max() + Wt, side="right")
        lo_c = int(lo_t) // M_CHUNK
        hi_c = min((int(hi_t) + M_CHUNK - 1) // M_CHUNK, m // M_CHUNK)
        # round to an even number of chunks (GROUP = 2 chunks), stay in range
        nch = hi_c - lo_c
        if nch % 2:
            if hi_c < m // M_CHUNK:
                hi_c += 1
            elif lo_c > 0:
                lo_c -= 1
            else:
                hi_c += 1           # pad beyond end; gather clips (dup cols)
        windows.append((lo_c, hi_c))
    return s_s, t_s, sn_s, windows


def _prepare_inputs(source_point_cloud, target_point_cloud):
    s_all = np.asarray(source_point_cloud, dtype=np.float32)
    t_all = np.asarray(target_point_cloud, dtype=np.float32)

    # plan per batch
    plans = []
    for b in range(B):
        s_s, t_s, sn_s, windows = _plan_batch(s_all[b], t_all[b])
        # flat group list: (tile_idx, chunk_lo) per GROUP (=2 chunks)
        groups = []
        for ti, (lo_c, hi_c) in enumerate(windows):
            for c in range(lo_c, hi_c, 2):
                groups.append((ti, c))
        plans.append({"s": s_s, "t": t_s, "groups": groups})

    g_per_core = max((len(p["groups"]) + CORES_PER_BATCH - 1) // CORES_PER_BATCH
                     for p in plans)
    G = max(g_per_core, 2)

    # build per-batch operand pieces
    batch_data = []
    for b in range(B):
        p = plans[b]
        s_s, t_s = p["s"], p["t"]
        sh, sl, sl2 = _split3(s_s)
        s2 = (s_s ** 2).sum(-1)          # fp64
        s2h, s2l, s2l2 = _split3(s2)
        th, tl, tl2 = _split3(t_s)
        t2 = (t_s ** 2).sum(-1)
        t2h, t2l, t2l2 = _split3(t2)

        # K x n lhs rows and K x m rhs rows (sorted order)
        nn_ = len(s_s); mm_ = len(t_s)
        lhs_rows = np.zeros((K, nn_), dtype=bf16)
        rhs_rows = np.zeros((K, mm_), dtype=bf16)

        def m2(x):
            return (np.float32(-2.0) * x.astype(np.float32)).astype(bf16)

        for d in range(D):
            lhs_rows[0 + d] = sh[:, d];  rhs_rows[0 + d] = m2(th[:, d])
            lhs_rows[3 + d] = sh[:, d];  rhs_rows[3 + d] = m2(tl[:, d])
            lhs_rows[6 + d] = sl[:, d];  rhs_rows[6 + d] = m2(th[:, d])
            lhs_rows[9 + d] = sl[:, d];  rhs_rows[9 + d] = m2(tl[:, d])
            lhs_rows[12 + d] = sh[:, d]; rhs_rows[12 + d] = m2(tl2[:, d])
            lhs_rows[15 + d] = sl2[:, d]; rhs_rows[15 + d] = m2(th[:, d])
        one_n = np.ones(nn_, dtype=bf16); one_m = np.ones(mm_, dtype=bf16)
        lhs_rows[18] = one_n; rhs_rows[18] = t2h
        lhs_rows[19] = one_n; rhs_rows[19] = t2l
        lhs_rows[20] = one_n; rhs_rows[20] = t2l2
        lhs_rows[21] = s2h;   rhs_rows[21] = one_m
        lhs_rows[22] = s2l;   rhs_rows[22] = one_m
        lhs_rows[23] = s2l2;  rhs_rows[23] = one_m

        s2_dev = (s2h.astype(np.float64) + s2l.astype(np.float64)
                  + s2l2.astype(np.float64))
        batch_data.append({
            "lhs_rows": lhs_rows, "rhs_rows": rhs_rows,
            "s2_resid": s2 - s2_dev, "groups": plans[b]["groups"],
            "m_chunks": mm_ // M_CHUNK,
        })

    # assign contiguous slabs of the flat group list to cores; pad with
    # duplicates of group 0 (host ignores padded columns)
    in_maps, core_maps = [], []
    for core in range(N_CORES):
        b = core // CORES_PER_BATCH
        q = core % CORES_PER_BATCH
        bd = batch_data[b]
        groups = bd["groups"]
        per = (len(groups) + CORES_PER_BATCH - 1) // CORES_PER_BATCH
        sel = groups[q * per:(q + 1) * per]
        pad = G - len(sel)
        sel_padded = sel + [groups[0]] * pad if sel else [groups[0]] * G

        lhs = np.zeros((64, G * 128), dtype=bf16)
        rhs = np.zeros((64, G * GROUP), dtype=bf16)
        mc = bd["m_chunks"]
        for gi, (ti, c) in enumerate(sel_padded):
            lhs[0:K, gi * 128:(gi + 1) * 128] = \
                bd["lhs_rows"][:, ti * 128:(ti + 1) * 128]
            c2 = min(c + 2, mc)          # clip; duplicate last chunk if needed
            cols = bd["rhs_rows"][:, c * M_CHUNK:c2 * M_CHUNK]
            if c2 - c < 2:
                cols = np.concatenate([cols, cols[:, :M_CHUNK]], axis=1)
            rhs[0:K, gi * GROUP:(gi + 1) * GROUP] = cols
        lhs[32:32 + K] = lhs[0:K]
        rhs[32:32 + K] = rhs[0:K]

        in_maps.append({"lhs": lhs, "rhs": rhs})
        core_maps.append({"batch": b, "sel": sel, "n_real": len(sel)})

    return G, in_maps, core_maps, batch_data


def _run(source_point_cloud, target_point_cloud, trace=False):
    G, in_maps, core_maps, batch_data = _prepare_inputs(
        source_point_cloud, target_point_cloud)
    nc = _get_nc(G)
    res = run_bass_kernel_spmd(nc, in_maps, core_ids=list(range(N_CORES)),
                               trace=trace)

    # host combine: per batch, min over each tile's group columns
    ntiles = N // 128
    best = [np.full((ntiles * 128,), np.inf) for _ in range(B)]
    for core in range(N_CORES):
        cm = core_maps[core]
        out = res.results[core]["out"].astype(np.float64)  # [128, G]
        bb = best[cm["batch"]]
        for gi, (ti, _c) in enumerate(cm["sel"]):
            rows = slice(ti * 128, (ti + 1) * 128)
            bb[rows] = np.minimum(bb[rows], out[:, gi])
    total = 0.0
    for b in range(B):
        total += best[b].sum() + batch_data[b]["s2_resid"].sum()
    loss = total / (B * N * D)
    return np.float32(loss), res


def kernel(source_point_cloud, target_point_cloud):
    out, _ = _run(source_point_cloud, target_point_cloud,
                  trace=bool(os.environ.get("BASS_TRACE")))
    return out
